# revision 1
# baseline (speedup 1.0000x reference)
"""Trainium2 Bass kernel for nn_BatchdenseGAT: 2-layer dense GAT, batch 16x512 nodes.

v2: data-parallel (2 graphs/core). Host packs all weights + adj^T + h^T in
matmul-ready bf16 layouts (single big DMAs). On device, per (layer, graph, head):
  - hp^T [o,n] from weights-stationary MMs; tanh on ACT.
  - scores: s as a psum row (stat=a_src*0.8), d as psum COLUMNS
    (stat=t-slices, mov=a_dst) -- no transposes.
  - E'[m,n] = adjT * max(exp(0.8s)[n]*exp(d)[m], exp(0.2d)[m]); the exp(0.8s)
    row is broadcast across partitions by a stride-0 DMA.
  - softmax denominators as psum COLUMNS via stat=E'-slices, mov=ones.
  - out matmul node-major: out[n,o] = sum_m E'[m,n] hp[m,o] (stat=E' slices,
    mov=hp node-major from a DMA-XBAR transpose). Normalization is then a
    per-partition tensor_scalar/scalar_tensor_tensor with 1/r columns.
  - elu(z) = min(exp(z)-1, relu(z)); layer-1 head-mean folded into the ones
    (x8) rowsum; log_softmax runs on the node-major accumulator, Ln batched
    last (one ACT table switch).
  - PE warmup matmuls during input DMA keep the tensor engine p-state high.
"""

import os
import sys
import numpy as np

B, N, V, D_EMB, F0, H = 16, 512, 100000, 64, 64, 8
O1 = O2 = 128
EPS = 1e-5
NCORES = 8
G = B // NCORES         # graphs per core = 2
NCH = N // 128          # 4 node chunks
NU = 2 * G * H          # 32 units per core

_cache = {}


def _ensure_paths():
    p = "/opt/trn_rl_repo/concourse"
    if os.path.isdir(p) and p not in sys.path:
        sys.path.append(p)


N_WARMUP = 14           # PE warmup matmuls (p-state ramp) during input DMA
PIPE = 4                # software pipeline depth (stageA leads stageB by PIPE)


def _build_nc():
    _ensure_paths()
    import concourse.bass as bass
    import concourse.tile as tile
    import concourse.mybir as mybir
    from concourse import bacc
    from contextlib import ExitStack

    F32 = mybir.dt.float32
    F32R = mybir.dt.float32r
    BF16 = mybir.dt.bfloat16
    I32 = mybir.dt.int32
    AX = mybir.AxisListType
    OP = mybir.AluOpType
    ACT = mybir.ActivationFunctionType

    nc = bacc.Bacc("TRN2", debug=False, enable_asserts=False)

    d_adjT = nc.dram_tensor("adjT", [128, G * NCH * N], BF16, kind="ExternalInput").ap()
    d_hT = nc.dram_tensor("hT", [F0, G * N], BF16, kind="ExternalInput").ap()
    d_ueT = nc.dram_tensor("ueT", [3, G * N], F32, kind="ExternalInput").ap()
    d_idx = nc.dram_tensor("idx", [128, G * NCH], I32, kind="ExternalInput").ap()
    d_emb = nc.dram_tensor("emb", [V, D_EMB], F32, kind="ExternalInput").ap()
    d_w0a = nc.dram_tensor("w0a", [128, H * 128], BF16, kind="ExternalInput").ap()
    d_w0b = nc.dram_tensor("w0b", [3, H * 128], BF16, kind="ExternalInput").ap()
    d_w1 = nc.dram_tensor("w1", [128, H * 8 * 128], BF16, kind="ExternalInput").ap()
    d_ap = nc.dram_tensor("apv", [128, 4 * H], BF16, kind="ExternalInput").ap()
    d_nw = nc.dram_tensor("nw", [D_EMB, 4], F32, kind="ExternalInput").ap()
    d_out = nc.dram_tensor("out", [G, N, O2], F32, kind="ExternalOutput").ap()

    with tile.TileContext(nc) as tc, ExitStack() as ctx:
        pers = ctx.enter_context(tc.tile_pool(name="pers", bufs=1))
        wk = ctx.enter_context(tc.tile_pool(name="wk", bufs=4))
        xb = ctx.enter_context(tc.tile_pool(name="xb", bufs=PIPE + 2))
        ep_pool = ctx.enter_context(tc.tile_pool(name="ep", bufs=4 * (PIPE + 1)))
        psA = ctx.enter_context(tc.tile_pool(name="psA", bufs=4, space="PSUM"))
        psB = ctx.enter_context(tc.tile_pool(name="psB", bufs=2, space="PSUM"))
        psS = ctx.enter_context(tc.tile_pool(name="psS", bufs=1, space="PSUM"))
        psC = ctx.enter_context(tc.tile_pool(name="psC", bufs=1, space="PSUM"))

        MM = nc.tensor.matmul

        # round-robin engine pickers for copies and for the adjT masking
        _cp = [0]

        def anycopy(out, in_):
            # psum-capable engines only (GPSIMD cannot access PSUM on hw)
            _cp[0] ^= 1
            if _cp[0]:
                nc.scalar.copy(out, in_)
            else:
                nc.vector.tensor_copy(out=out, in_=in_)

        # ---------- persistents + input DMAs (big, few, split across queues) ----------
        adjT = pers.tile([128, G * NCH * N], BF16, tag="adjT")
        xTa = pers.tile([128, G * N], BF16, tag="xTa")
        xTb = pers.tile([3, G * N], BF16, tag="xTb")
        ueT = pers.tile([3, G * N], F32, tag="ueT")
        x1T = pers.tile([128, G * 8 * N], BF16, tag="x1T")
        accn = pers.tile([128, G * NCH * 128], F32, tag="accn")
        w0a = pers.tile([128, H * 128], BF16, tag="w0a")
        w0b = pers.tile([3, H * 128], BF16, tag="w0b")
        w1 = pers.tile([128, H * 8 * 128], BF16, tag="w1")
        apv = pers.tile([128, 4 * H], BF16, tag="apv")
        nwb = pers.tile([D_EMB, 4], F32, tag="nwb")
        idx = pers.tile([128, G * NCH], I32, tag="idx")
        ones_c = pers.tile([128, 2], BF16, tag="ones_c")
        negone = pers.tile([128, 1], F32, tag="negone")
        zeros_w = pers.tile([128, N], BF16, tag="zeros_w")

        nc.vector.memset(ones_c[:, 0:1], 1.0)
        nc.vector.memset(negone[:], -1.0)
        nc.vector.memset(ones_c[:, 1:2], float(H))
        nc.vector.memset(zeros_w[:], 0.0)

        nc.sync.dma_start(idx[:], d_idx[:])
        nc.scalar.dma_start(nwb[:], d_nw[:])
        nc.sync.dma_start(ueT[:], d_ueT[:])
        nc.sync.dma_start(xTa[0:F0, :], d_hT[:])
        nc.scalar.dma_start(w0a[:], d_w0a[:])
        nc.scalar.dma_start(w0b[:], d_w0b[:])
        nc.scalar.dma_start(apv[:], d_ap[:])
        nc.sync.dma_start(adjT[:], d_adjT[:])
        nc.scalar.dma_start(w1[:], d_w1[:])

        # ---------- PE warmup: keep p-state high while DMAs land ----------
        for i in range(N_WARMUP):
            wu = psS.tile([33, N], F32, tag="sd", name=f"wu{i}")
            MM(wu[0:1, :], zeros_w[:, 0:1], zeros_w[:], start=True, stop=True)

        # ---------- per-graph preprocessing ----------
        def newton_rsqrt(vare, P):
            iv = vare[:].bitcast(mybir.dt.int32)
            sh = wk.tile([P, 1], mybir.dt.int32, tag="in_sh")
            nc.vector.tensor_scalar(sh[:], iv, 1, None, OP.arith_shift_right)
            y = wk.tile([P, 1], F32, tag="in_y")
            nc.vector.tensor_scalar(y[:].bitcast(mybir.dt.int32), sh[:], -1,
                                    0x5f3759df, OP.mult, OP.add)
            rstd = y
            for it in range(3):
                y2 = wk.tile([P, 1], F32, tag="in_y2", name=f"y2{it}")
                nc.vector.tensor_tensor(out=y2[:], in0=rstd[:], in1=rstd[:], op=OP.mult)
                vy2 = wk.tile([P, 1], F32, tag="in_vy2", name=f"vy2{it}")
                nc.vector.tensor_tensor(out=vy2[:], in0=vare[:], in1=y2[:], op=OP.mult)
                corr = wk.tile([P, 1], F32, tag="in_corr", name=f"corr{it}")
                nc.vector.tensor_scalar(corr[:], vy2[:], -0.5, 1.5, OP.mult, OP.add)
                ynew = wk.tile([P, 1], F32, tag="in_ynew", name=f"ynew{it}")
                nc.vector.tensor_tensor(out=ynew[:], in0=rstd[:], in1=corr[:], op=OP.mult)
                rstd = ynew
            return rstd

        ident = pers.tile([128, 128], F32, tag="ident")
        ident_b = pers.tile([128, 128], BF16, tag="ident_b")
        from concourse.masks import make_identity
        make_identity(nc, ident[:])
        make_identity(nc, ident_b[:])

        def norm_stats(src, P, sums, col):
            nc.vector.tensor_reduce(sums[0][:, col:col + 1], src, AX.X, OP.add)
            sq = wk.tile([P, N], BF16, tag="in_sq", name=f"sq{col}")
            nc.scalar.activation(sq[:], src, ACT.Square,
                                 accum_out=sums[1][:, col:col + 1])

        def norm_finish(srcs, P, sums, w_col, b_col, dsts):
            mu = wk.tile([P, G], F32, tag="in_mu")
            nc.vector.tensor_scalar(mu[:], sums[0][:], 1.0 / N, None, OP.mult)
            ex2 = wk.tile([P, G], F32, tag="in_ex2")
            nc.vector.tensor_scalar(ex2[:], sums[1][:], 1.0 / N, None, OP.mult)
            musq = wk.tile([P, G], F32, tag="in_musq")
            nc.vector.tensor_tensor(out=musq[:], in0=mu[:], in1=mu[:], op=OP.mult)
            vare = wk.tile([P, G], F32, tag="in_vare")
            nc.vector.tensor_tensor(out=vare[:], in0=ex2[:], in1=musq[:], op=OP.subtract)
            nc.vector.tensor_scalar(vare[:], vare[:], EPS, None, OP.add)
            iv = vare[:].bitcast(mybir.dt.int32)
            sh = wk.tile([P, G], mybir.dt.int32, tag="in_sh")
            nc.vector.tensor_scalar(sh[:], iv, 1, None, OP.arith_shift_right)
            y = wk.tile([P, G], F32, tag="in_y")
            nc.vector.tensor_scalar(y[:].bitcast(mybir.dt.int32), sh[:], -1,
                                    0x5f3759df, OP.mult, OP.add)
            rstd = y
            for it in range(2):
                y2 = wk.tile([P, G], F32, tag="in_y2", name=f"y2{it}")
                nc.vector.tensor_tensor(out=y2[:], in0=rstd[:], in1=rstd[:], op=OP.mult)
                vy2 = wk.tile([P, G], F32, tag="in_vy2", name=f"vy2{it}")
                nc.vector.tensor_tensor(out=vy2[:], in0=vare[:], in1=y2[:], op=OP.mult)
                corr = wk.tile([P, G], F32, tag="in_corr", name=f"corr{it}")
                nc.vector.tensor_scalar(corr[:], vy2[:], -0.5, 1.5, OP.mult, OP.add)
                ynew = wk.tile([P, G], F32, tag="in_ynew", name=f"ynew{it}")
                nc.vector.tensor_tensor(out=ynew[:], in0=rstd[:], in1=corr[:], op=OP.mult)
                rstd = ynew
            scl = wk.tile([P, G], F32, tag="in_scl")
            nc.vector.tensor_scalar(scl[:], rstd[:], w_col, None, OP.mult)
            tb = wk.tile([P, G], F32, tag="in_tb")
            nc.vector.tensor_tensor(out=tb[:], in0=mu[:], in1=scl[:], op=OP.mult)
            bia = wk.tile([P, G], F32, tag="in_bia")
            nc.vector.tensor_scalar(bia[:], tb[:], -1.0, b_col, OP.mult, OP.add)
            for g in range(G):
                nc.vector.tensor_scalar(dsts[g], srcs[g], scl[:, g:g + 1],
                                        bia[:, g:g + 1], OP.mult, OP.add)

        def prep_all():
            embTs = []
            es0 = pers.tile([D_EMB, G], F32, tag="es0", name="es0")
            es1 = pers.tile([D_EMB, G], F32, tag="es1", name="es1")
            us0 = pers.tile([3, G], F32, tag="us0", name="us0")
            us1 = pers.tile([3, G], F32, tag="us1", name="us1")
            esums = (es0, es1)
            usums = (us0, us1)
            for g in range(G):
                embT = psB.tile([D_EMB, N], F32, tag="out", name=f"embT{g}")
                for i in range(NCH):
                    gat = wk.tile([128, D_EMB], F32, tag="gat", bufs=2 * NCH,
                                  name=f"gat{g}_{i}")
                    nc.gpsimd.indirect_dma_start(
                        out=gat[:], out_offset=None, in_=d_emb[:],
                        in_offset=bass.IndirectOffsetOnAxis(
                            ap=idx[:, g * NCH + i:g * NCH + i + 1], axis=0))
                    nc.tensor.transpose(embT[:, 128 * i:128 * (i + 1)], gat[:], ident[:])
                embTs.append(embT)
                norm_stats(embT[:], D_EMB, esums, g)
                norm_stats(ueT[0:3, g * N:(g + 1) * N], 3, usums, g)
            norm_finish([embTs[g][:] for g in range(G)], D_EMB, esums,
                        nwb[:, 0:1], nwb[:, 1:2],
                        [xTa[F0:128, g * N:(g + 1) * N] for g in range(G)])
            norm_finish([ueT[0:3, g * N:(g + 1) * N] for g in range(G)], 3, usums,
                        nwb[0:3, 2:3], nwb[0:3, 3:4],
                        [xTb[:, g * N:(g + 1) * N] for g in range(G)])

        # ---------- unit stages ----------        # ---------- unit stages ----------        # ---------- unit stages ----------
        def a_col(layer, h, role):
            return apv[:, 4 * h + 2 * layer + role: 4 * h + 2 * layer + role + 1]

        def stage1(layer, g, h):
            gofs = g * N
            hp_fm = psA.tile([128, N], F32, tag="hp", name="hp_fm")
            if layer == 0:
                MM(hp_fm[:], w0a[:, 128 * h:128 * (h + 1)], xTa[:, gofs:gofs + N],
                   start=True, stop=False)
                MM(hp_fm[:], w0b[:, 128 * h:128 * (h + 1)], xTb[:, gofs:gofs + N],
                   start=False, stop=True)
            else:
                for k in range(8):
                    MM(hp_fm[:], w1[:, (h * 8 + k) * 128:(h * 8 + k + 1) * 128],
                       x1T[:, (g * 8 + k) * N:(g * 8 + k) * N + N],
                       start=(k == 0), stop=(k == 7))
            # consume hp_fm promptly (psA stays 2-deep): tanh + bf16 copy
            t_sb = wk.tile([128, N], BF16, tag="tt", name="t_sb")
            nc.scalar.activation(t_sb[:], hp_fm[:], ACT.Tanh)
            hp_sb = wk.tile([128, N], BF16, tag="hpsb", name="hp_sb")
            anycopy(hp_sb[:], hp_fm[:])
            return dict(layer=layer, g=g, h=h, t_sb=t_sb, hp_sb=hp_sb)

        def stage2a(st):
            layer, g, h = st["layer"], st["g"], st["h"]
            t_sb, hp_sb = st["t_sb"], st["hp_sb"]
            # hp node-major via PE transposes into one psum tile + strided copy
            tp = psA.tile([128, N], BF16, tag="hp", name="tpA")
            for j in range(NCH):
                nc.tensor.transpose(tp[:, 128 * j:128 * (j + 1)],
                                    hp_sb[:, 128 * j:128 * (j + 1)], ident_b[:])
            hp_nm = xb.tile([128, 4 * 129], BF16, tag="hpnm", name="hp_nm")
            dst_v = hp_nm[:].rearrange("p (k c) -> p k c", k=NCH)[:, :, 0:128]
            src_v = tp[:].rearrange("p (k c) -> p k c", k=NCH)
            anycopy(dst_v, src_v)
            nc.gpsimd.memset(hp_nm[:, 128::129], 1.0)
            # score rows: s=0.8*a_src@t at partition 0, d=a_dst@t at partition 32
            sd = psS.tile([33, N], F32, tag="sd", name="sd")
            MM(sd[0:1, :], a_col(layer, h, 0), t_sb[:], start=True, stop=True)
            MM(sd[32:33, :], a_col(layer, h, 1), t_sb[:], start=True, stop=True)
            # d columns (for the leaky branch scalar)
            cols = psC.tile([128, 4], F32, tag="cols", name="cols")
            for j in range(NCH):
                MM(cols[:, j:j + 1], t_sb[:, 128 * j:128 * (j + 1)],
                   a_col(layer, h, 1), start=True, stop=True)
            prow = wk.tile([1, N], F32R, tag="prow", name="prow")
            nc.scalar.activation(prow[:], sd[0:1, :], ACT.Exp)
            qrow = wk.tile([1, N], F32R, tag="qrow", name="qrow")
            nc.scalar.activation(qrow[:], sd[32:33, :], ACT.Exp)
            acol = wk.tile([128, 4], F32, tag="acol", name="acol")
            nc.scalar.activation(acol[:], cols[:], ACT.Exp, scale=0.2)
            st["prow"] = prow
            st["qrow"] = qrow
            st["acol"] = acol
            st["hp_nm"] = hp_nm
            return st

        def stage2b(st):
            g = st["g"]
            prow, qrow, acol = st["prow"], st["qrow"], st["acol"]
            # E' tiles: pbq = exp(d[m]) (x) exp(0.8 s[n]) via PE rank-1 outer,
            # then ONE fused op: e = max(pbq, exp(0.2 d)) * adjT
            eps_ = []
            for j in range(NCH):
                pbq = psA.tile([128, N], F32, tag="hp", name=f"pbq{j}")
                MM(pbq[:], qrow[0:1, 128 * j:128 * (j + 1)], prow[:],
                   start=True, stop=True)
                e = ep_pool.tile([128, N], BF16, tag="ep", name=f"e{j}")
                nc.vector.scalar_tensor_tensor(
                    out=e[:], in0=pbq[:], scalar=acol[:, j:j + 1],
                    in1=adjT[:, (g * NCH + j) * N:(g * NCH + j + 1) * N],
                    op0=OP.max, op1=OP.mult)
                eps_.append(e)
            st["eps"] = eps_
            return st

        def stage3(st):
            layer, g, h = st["layer"], st["g"], st["h"]
            hp_nm, eps_ = st["hp_nm"], st["eps"]
            # out[n,o] blocks with a fused ones-column: col 128 of each 129-wide
            # block is the softmax denominator for that n-chunk
            ot = [psB.tile([128, 258], F32, tag="out", name=f"ot{half}")
                  for half in range(2)]
            for j in range(NCH):
                dst = ot[j // 2][:, (j % 2) * 129:(j % 2) * 129 + 129]
                for k in range(NCH):
                    MM(dst, eps_[k][:, 128 * j:128 * (j + 1)],
                       hp_nm[:, k * 129:k * 129 + 129],
                       start=(k == 0), stop=(k == NCH - 1))
            rcol = xb.tile([128, 4], F32, tag="rcol", name="rcol")
            nc.vector.reciprocal_approx_fast(out=rcol[:, 0:2], in_=ot[0][:, 128::129])
            nc.vector.reciprocal_approx_fast(out=rcol[:, 2:4], in_=ot[1][:, 128::129])
            if layer == 0:
                # z = out * (1/r) per n-chunk, fused into ACT identity w/ scale
                z = xb.tile([128, N], BF16, tag="z", name="z")
                for j in range(NCH):
                    nc.scalar.activation(z[:, 128 * j:128 * (j + 1)],
                                         ot[j // 2][:, (j % 2) * 129:(j % 2) * 129 + 128],
                                         ACT.Identity, scale=rcol[:, j:j + 1])
                st["z"] = z
            else:
                # fold the mean-over-heads 1/8 into the reciprocal columns
                rcol8 = xb.tile([128, 4], F32, tag="rcol8", name="rcol8")
                nc.vector.tensor_scalar(rcol8[:], rcol[:], 0.125, None, OP.mult)
                base = g * NCH * 128
                for j in range(NCH):
                    dst = accn[:, base + 128 * j:base + 128 * (j + 1)]
                    src_ap = ot[j // 2][:, (j % 2) * 129:(j % 2) * 129 + 128]
                    if h == 0:
                        nc.vector.tensor_scalar(dst, src_ap,
                                                rcol8[:, j:j + 1], None, OP.mult)
                    else:
                        nc.vector.scalar_tensor_tensor(
                            out=dst, in0=src_ap,
                            scalar=rcol8[:, j:j + 1], in1=dst,
                            op0=OP.mult, op1=OP.add)

        def stage4(st):
            # deferred l0 tail: elu + x1 transposes (off every critical path)
            if st["layer"] != 0:
                return
            g, h, z = st["g"], st["h"], st["z"]
            ez1 = wk.tile([128, N], BF16, tag="ez1", name="ez1")
            nc.scalar.activation(ez1[:], z[:], ACT.Exp)
            nc.scalar.activation(ez1[:], ez1[:], ACT.Identity, bias=negone[:])
            r1m = wk.tile([128, N], BF16, tag="r1m", name="r1m")
            nc.scalar.activation(r1m[:], z[:], ACT.Relu)
            x1n = wk.tile([128, N], BF16, tag="x1n", name="x1n")
            # elu(z) = min(exp(z) - 1, relu(z))
            nc.vector.tensor_tensor(out=x1n[:], in0=ez1[:], in1=r1m[:], op=OP.min)
            tp = psA.tile([128, N], BF16, tag="hp", name="tpB")
            for j in range(NCH):
                nc.tensor.transpose(tp[:, 128 * j:128 * (j + 1)],
                                    x1n[:, 128 * j:128 * (j + 1)], ident_b[:])
            base = (g * 8 + h) * N
            anycopy(x1T[:, base:base + N], tp[:])

        # ---------- epilogue part 1 (exp domain; per graph) ----------
        nmax_all = pers.tile([128, G * NCH], F32, tag="nmax_all")
        sexp_all = pers.tile([128, G * NCH], F32, tag="sexp_all")

        def epilogue_exp(g):
            for j in range(NCH):
                c = g * NCH + j
                blk = accn[:, c * 128:(c + 1) * 128]
                nc.vector.tensor_reduce(nmax_all[:, c:c + 1], blk, AX.X, OP.max,
                                        negate=True)
                esc = wk.tile([128, 128], BF16, tag="esc", name="esc")
                nc.scalar.activation(esc[:], blk, ACT.Exp, bias=nmax_all[:, c:c + 1],
                                     accum_out=sexp_all[:, c:c + 1])

        # ---------- pipeline: S1(i) || S2a(i-2) || S2b(i-3) || S3(i-5) || S4(i-7) ----------
        units = [(l, g, h) for l in range(2) for g in range(G) for h in range(H)]
        prep_all()
        for i in range(10):
            wu = psS.tile([33, N], F32, tag="sd", name=f"wu2_{i}")
            MM(wu[0:1, :], zeros_w[:, 0:1], zeros_w[:], start=True, stop=True)
        NUx = len(units)
        sts = [None] * NUx
        L2A, L2B, L3, L4 = 2, 3, 5, 7
        for i in range(NUx + L4):
            if i < NUx:
                sts[i] = stage1(*units[i])
            if L2A <= i and i - L2A < NUx:
                stage2a(sts[i - L2A])
            if L2B <= i and i - L2B < NUx:
                stage2b(sts[i - L2B])
            if L3 <= i and i - L3 < NUx:
                st = sts[i - L3]
                stage3(st)
                if st["layer"] == 1 and st["h"] == H - 1:
                    epilogue_exp(st["g"])
            if L4 <= i and i - L4 < NUx:
                stage4(sts[i - L4])

        # ---------- epilogue part 2        # ---------- epilogue part 2        # ---------- epilogue part 2        # ---------- epilogue part 2        # ---------- epilogue part 2: ONE Ln (one table switch) + final add ----------
        lns = pers.tile([128, G * NCH], F32, tag="lns")
        nc.scalar.activation(lns[:], sexp_all[:], ACT.Ln)
        cc = pers.tile([128, G * NCH], F32, tag="cc")
        nc.vector.tensor_tensor(out=cc[:], in0=nmax_all[:], in1=lns[:],
                                op=OP.subtract)
        for g in range(G):
            fin = wk.tile([128, 4 * 128], F32, tag="fin", bufs=2, name="fin")
            for j in range(NCH):
                i = g * NCH + j
                blk = accn[:, i * 128:(i + 1) * 128]
                if j % 2 == 0:
                    nc.vector.tensor_scalar(fin[:, j * 128:(j + 1) * 128], blk,
                                            cc[:, i:i + 1], None, OP.add)
                else:
                    nc.scalar.activation(fin[:, j * 128:(j + 1) * 128], blk,
                                         ACT.Identity, bias=cc[:, i:i + 1])
            # one contiguous 256KB DMA per graph: [n_local, (j, o)] -> [j, n_local, o]
            fin_v = fin[:].rearrange("p (j o) -> p j o", j=NCH)
            nc.sync.dma_start(d_out[g].rearrange("(j p) o -> p j o", j=NCH), fin_v)

    nc.finalize()
    return nc


def _get_nc():
    if "nc" not in _cache:
        _cache["nc"] = _build_nc()
    return _cache["nc"]


def shard_inputs(inputs):
    """Full inputs -> list of 8 per-core input maps (pure layout/dtype prep)."""
    import ml_dtypes
    bf16 = ml_dtypes.bfloat16

    adj = np.asarray(inputs["adj"], dtype=np.float32)
    h = np.asarray(inputs["h"], dtype=np.float32)
    ue = np.asarray(inputs["user_emb"], dtype=np.float32)
    emb = np.ascontiguousarray(np.asarray(inputs["emb_table"], dtype=np.float32))
    vert = np.asarray(inputs["vertices"]).astype(np.int32)

    w0 = np.asarray(inputs["w0"], dtype=np.float32)
    w0a = np.ascontiguousarray(
        w0[:, :128, :].transpose(1, 0, 2).reshape(128, H * 128).astype(bf16))
    w0b = np.ascontiguousarray(
        w0[:, 128:131, :].transpose(1, 0, 2).reshape(3, H * 128).astype(bf16))
    w1 = np.asarray(inputs["w1"], dtype=np.float32)
    w1p = np.ascontiguousarray(
        w1.reshape(H, 8, 128, 128).transpose(2, 0, 1, 3).reshape(128, H * 8 * 128)
        .astype(bf16))
    # a-vector columns: [h*4 + {0: 0.8*a_src0, 1: a_dst0, 2: 0.8*a_src1, 3: a_dst1}]
    apv = np.zeros((128, 4 * H), np.float32)
    for h_ in range(H):
        apv[:, 4 * h_ + 0] = 0.8 * np.asarray(inputs["a_src0"])[h_, :, 0]
        apv[:, 4 * h_ + 1] = np.asarray(inputs["a_dst0"])[h_, :, 0]
        apv[:, 4 * h_ + 2] = 0.8 * np.asarray(inputs["a_src1"])[h_, :, 0]
        apv[:, 4 * h_ + 3] = np.asarray(inputs["a_dst1"])[h_, :, 0]
    apv = apv.astype(bf16)
    nw = np.zeros((D_EMB, 4), np.float32)
    nw[:, 0] = np.asarray(inputs["norm1_w"], dtype=np.float32)
    nw[:, 1] = np.asarray(inputs["norm1_b"], dtype=np.float32)
    nw[0:3, 2] = np.asarray(inputs["norm2_w"], dtype=np.float32)
    nw[0:3, 3] = np.asarray(inputs["norm2_b"], dtype=np.float32)

    maps = []
    for c in range(NCORES):
        sl = slice(G * c, G * (c + 1))
        adjT = adj[sl].transpose(0, 2, 1).reshape(G, NCH, 128, N) \
            .transpose(2, 0, 1, 3).reshape(128, G * NCH * N).astype(bf16)
        hT = h[sl].transpose(0, 2, 1).transpose(1, 0, 2).reshape(F0, G * N).astype(bf16)
        ueT = ue[sl].transpose(0, 2, 1).transpose(1, 0, 2).reshape(3, G * N)
        idxp = vert[sl].reshape(G, NCH, 128).transpose(2, 0, 1).reshape(128, G * NCH)
        maps.append({
            "adjT": np.ascontiguousarray(adjT),
            "hT": np.ascontiguousarray(hT),
            "ueT": np.ascontiguousarray(ueT.astype(np.float32)),
            "idx": np.ascontiguousarray(idxp),
            "emb": emb,
            "w0a": w0a, "w0b": w0b, "w1": w1p, "apv": apv, "nw": nw,
        })
    return maps


def kernel(**inputs):
    _ensure_paths()
    from concourse import bass_utils
    nc = _get_nc()
    maps = shard_inputs(inputs)
    res = bass_utils.run_bass_kernel_spmd(nc, maps, core_ids=list(range(NCORES)))
    out = np.concatenate([res.results[c]["out"] for c in range(NCORES)], axis=0)
    return out



# revision 33
# speedup vs baseline: 1.2752x; 1.2752x over previous
"""Trainium2 Bass kernel for nn_BatchdenseGAT: 2-layer dense GAT, batch 16x512 nodes.

v3: data-parallel (2 graphs/core), 16 (layer,head) pairs each covering both
graphs (stationary weights shared across graphs). Major changes vs v2:
  - exp(0.8 s) is produced BROADCAST across partitions by a matmul whose
    stationary is 0.8*a_src replicated into 128 columns (sbc psum -> one ACT
    exp). No [1,n] rows, no PE rank-1 outer products.
  - E'[m,n] = adjT * max(pcb*exp(d[m]), exp(0.2 d[m])) built on DVE:
    tensor_scalar (mult,max; 4x mode) + tensor_tensor mask-mult (2x mode).
  - all hp/x1 transposes go through DMA-XBAR (dma_start_transpose), PE does
    zero transposes in the main loop.
  - elu(z)-1 fold: layer-1 consumes x1' = elu(z)+1; the -1 is folded into a
    per-feature bias (host-precomputed w1 column sums) applied in tanh/copy.
    L0 tail: x1' = min(relu(z)+1, exp(z)) = 2 cheap ops + one gpsimd stt.
  - out matmul keeps node-major layout with the fused ones-column (softmax
    denominator at column 128 of each 129-wide mov block).
  - software pipeline issues oldest-stage-first per slot so no engine queue
    head-of-line blocks; 3 drain slots between layer 0 and layer 1.
"""

import os
import sys
import numpy as np

B, N, V, D_EMB, F0, H = 16, 512, 100000, 64, 64, 8
O1 = O2 = 128
EPS = 1e-5
NCORES = 8
G = B // NCORES         # graphs per core = 2
NCH = N // 128          # 4 node chunks

_cache = {}


def _ensure_paths():
    p = "/opt/trn_rl_repo/concourse"
    if os.path.isdir(p) and p not in sys.path:
        sys.path.append(p)


N_WARMUP = 14           # PE warmup matmuls (p-state ramp) during input DMA
BUB = 3                 # drain slots between layer 0 and layer 1
DBG = os.environ.get("KDBG", "0") == "1"


def _build_nc():
    _ensure_paths()
    import concourse.bass as bass
    import concourse.tile as tile
    import concourse.mybir as mybir
    from concourse import bacc
    from contextlib import ExitStack

    F32 = mybir.dt.float32
    BF16 = mybir.dt.bfloat16
    I32 = mybir.dt.int32
    AX = mybir.AxisListType
    OP = mybir.AluOpType
    ACT = mybir.ActivationFunctionType

    nc = bacc.Bacc("TRN2", debug=False, enable_asserts=False)

    d_adjT = nc.dram_tensor("adjT", [128, G * NCH * N], BF16, kind="ExternalInput").ap()
    d_hT = nc.dram_tensor("hT", [F0, G * N], BF16, kind="ExternalInput").ap()
    d_ueT = nc.dram_tensor("ueT", [3, G * N], F32, kind="ExternalInput").ap()
    d_idx = nc.dram_tensor("idx", [128, G * NCH], I32, kind="ExternalInput").ap()
    d_emb = nc.dram_tensor("emb", [V, D_EMB], F32, kind="ExternalInput").ap()
    d_w0a = nc.dram_tensor("w0a", [128, H * 128], BF16, kind="ExternalInput").ap()
    d_w0b = nc.dram_tensor("w0b", [3, H * 128], BF16, kind="ExternalInput").ap()
    d_w1 = nc.dram_tensor("w1", [128, H * 8 * 128], BF16, kind="ExternalInput").ap()
    d_asr = nc.dram_tensor("asr", [128, 2 * H * 128], BF16, kind="ExternalInput").ap()
    d_adc = nc.dram_tensor("adc", [128, 2 * H], BF16, kind="ExternalInput").ap()
    d_ncb = nc.dram_tensor("ncb", [128, H], F32, kind="ExternalInput").ap()
    d_nw = nc.dram_tensor("nw", [D_EMB, 4], F32, kind="ExternalInput").ap()
    d_out = nc.dram_tensor("out", [G, N, O2], F32, kind="ExternalOutput").ap()
    if DBG:
        d_dbg_x1T = nc.dram_tensor("dbg_x1T", [128, G * 8 * N], BF16,
                                   kind="ExternalOutput").ap()
        d_dbg_accn = nc.dram_tensor("dbg_accn", [128, G * NCH * 128], BF16,
                                    kind="ExternalOutput").ap()
        d_dbg_hpnm = nc.dram_tensor("dbg_hpnm", [128, NCH * 129], BF16,
                                    kind="ExternalOutput").ap()
        d_dbg_e = nc.dram_tensor("dbg_e", [128, NCH * N], BF16,
                                 kind="ExternalOutput").ap()
        d_dbg_pcb = nc.dram_tensor("dbg_pcb", [128, N], BF16,
                                   kind="ExternalOutput").ap()
        d_dbg_qa = nc.dram_tensor("dbg_qa", [128, 2 * NCH], F32,
                                  kind="ExternalOutput").ap()
        d_dbg_t = nc.dram_tensor("dbg_t", [128, N], BF16,
                                 kind="ExternalOutput").ap()
        d_dbg_xta = nc.dram_tensor("dbg_xta", [128, G * N], BF16,
                                   kind="ExternalOutput").ap()

    with tile.TileContext(nc) as tc, ExitStack() as ctx:
        pers = ctx.enter_context(tc.tile_pool(name="pers", bufs=1))
        wk = ctx.enter_context(tc.tile_pool(name="wk", bufs=4))
        xb = ctx.enter_context(tc.tile_pool(name="xb", bufs=24))
        ep_pool = ctx.enter_context(tc.tile_pool(name="ep", bufs=40))
        psA = ctx.enter_context(tc.tile_pool(name="psA", bufs=2, space="PSUM"))
        psS = ctx.enter_context(tc.tile_pool(name="psS", bufs=2, space="PSUM"))
        psB = ctx.enter_context(tc.tile_pool(name="psB", bufs=3, space="PSUM"))
        psC = ctx.enter_context(tc.tile_pool(name="psC", bufs=1, space="PSUM"))

        MM = nc.tensor.matmul

        # ---------- persistents + input DMAs ----------
        adjT = pers.tile([128, G * NCH * N], BF16, tag="adjT")
        xTa = pers.tile([128, G * N], BF16, tag="xTa")
        xTb = pers.tile([3, G * N], BF16, tag="xTb")
        ueT = pers.tile([3, G * N], F32, tag="ueT")
        x1seg = [[pers.tile([128, N], BF16, tag=f"x1seg{g}_{k}",
                            name=f"x1seg{g}_{k}")
                  for k in range(8)] for g in range(G)]
        accn = pers.tile([128, G * NCH * 128], BF16, tag="accn")
        w0a = pers.tile([128, H * 128], BF16, tag="w0a")
        w0b = pers.tile([3, H * 128], BF16, tag="w0b")
        w1 = pers.tile([128, H * 8 * 128], BF16, tag="w1")
        asr = pers.tile([128, 2 * H * 128], BF16, tag="asr")
        adc = pers.tile([128, 2 * H], BF16, tag="adc")
        ncb = pers.tile([128, H], F32, tag="ncb")
        nwb = pers.tile([D_EMB, 4], F32, tag="nwb")
        idx = pers.tile([128, G * NCH], I32, tag="idx")
        zeros_w = pers.tile([128, N], BF16, tag="zeros_w")

        nc.vector.memset(zeros_w[:], 0.0)

        nc.sync.dma_start(idx[:], d_idx[:])
        nc.scalar.dma_start(nwb[:], d_nw[:])
        nc.sync.dma_start(ueT[:], d_ueT[:])
        nc.sync.dma_start(xTa[0:F0, :], d_hT[:])
        nc.scalar.dma_start(w0a[:], d_w0a[:])
        nc.scalar.dma_start(w0b[:], d_w0b[:])
        nc.scalar.dma_start(asr[:], d_asr[:])
        nc.scalar.dma_start(adc[:], d_adc[:])
        nc.scalar.dma_start(ncb[:], d_ncb[:])
        nc.sync.dma_start(adjT[:], d_adjT[:])
        nc.scalar.dma_start(w1[:], d_w1[:])

        # ---------- PE warmup: keep p-state high while DMAs land ----------
        for i in range(N_WARMUP):
            wu = psA.tile([128, N], F32, tag="hp", name=f"wu{i}")
            MM(wu[0:1, :], zeros_w[:, 0:1], zeros_w[:], start=True, stop=True)

        # ---------- per-graph preprocessing (instance norms + embed) ----------
        ident = pers.tile([128, 128], F32, tag="ident")
        ident_b = pers.tile([128, 128], BF16, tag="ident_b")
        from concourse.masks import make_identity
        make_identity(nc, ident[:])
        make_identity(nc, ident_b[:])

        def norm_stats(src, P, sums, col):
            nc.vector.tensor_reduce(sums[0][:, col:col + 1], src, AX.X, OP.add)
            sq = wk.tile([P, N], BF16, tag="in_sq", name=f"sq{col}")
            nc.scalar.activation(sq[:], src, ACT.Square,
                                 accum_out=sums[1][:, col:col + 1])

        def norm_finish(srcs, P, sums, w_col, b_col, dsts):
            mu = wk.tile([P, G], F32, tag="in_mu")
            nc.vector.tensor_scalar(mu[:], sums[0][:], 1.0 / N, None, OP.mult)
            ex2 = wk.tile([P, G], F32, tag="in_ex2")
            nc.vector.tensor_scalar(ex2[:], sums[1][:], 1.0 / N, None, OP.mult)
            musq = wk.tile([P, G], F32, tag="in_musq")
            nc.vector.tensor_tensor(out=musq[:], in0=mu[:], in1=mu[:], op=OP.mult)
            vare = wk.tile([P, G], F32, tag="in_vare")
            nc.vector.tensor_tensor(out=vare[:], in0=ex2[:], in1=musq[:], op=OP.subtract)
            nc.vector.tensor_scalar(vare[:], vare[:], EPS, None, OP.add)
            iv = vare[:].bitcast(mybir.dt.int32)
            sh = wk.tile([P, G], mybir.dt.int32, tag="in_sh")
            nc.vector.tensor_scalar(sh[:], iv, 1, None, OP.arith_shift_right)
            y = wk.tile([P, G], F32, tag="in_y")
            nc.vector.tensor_scalar(y[:].bitcast(mybir.dt.int32), sh[:], -1,
                                    0x5f3759df, OP.mult, OP.add)
            rstd = y
            for it in range(2):
                y2 = wk.tile([P, G], F32, tag="in_y2", name=f"y2{it}")
                nc.vector.tensor_tensor(out=y2[:], in0=rstd[:], in1=rstd[:], op=OP.mult)
                vy2 = wk.tile([P, G], F32, tag="in_vy2", name=f"vy2{it}")
                nc.vector.tensor_tensor(out=vy2[:], in0=vare[:], in1=y2[:], op=OP.mult)
                corr = wk.tile([P, G], F32, tag="in_corr", name=f"corr{it}")
                nc.vector.tensor_scalar(corr[:], vy2[:], -0.5, 1.5, OP.mult, OP.add)
                ynew = wk.tile([P, G], F32, tag="in_ynew", name=f"ynew{it}")
                nc.vector.tensor_tensor(out=ynew[:], in0=rstd[:], in1=corr[:], op=OP.mult)
                rstd = ynew
            scl = wk.tile([P, G], F32, tag="in_scl")
            nc.vector.tensor_scalar(scl[:], rstd[:], w_col, None, OP.mult)
            tb = wk.tile([P, G], F32, tag="in_tb")
            nc.vector.tensor_tensor(out=tb[:], in0=mu[:], in1=scl[:], op=OP.mult)
            bia = wk.tile([P, G], F32, tag="in_bia")
            nc.vector.tensor_scalar(bia[:], tb[:], -1.0, b_col, OP.mult, OP.add)
            for g in range(G):
                nc.vector.tensor_scalar(dsts[g], srcs[g], scl[:, g:g + 1],
                                        bia[:, g:g + 1], OP.mult, OP.add)

        def prep_all():
            embTs = []
            es0 = pers.tile([D_EMB, G], F32, tag="es0", name="es0")
            es1 = pers.tile([D_EMB, G], F32, tag="es1", name="es1")
            us0 = pers.tile([3, G], F32, tag="us0", name="us0")
            us1 = pers.tile([3, G], F32, tag="us1", name="us1")
            esums = (es0, es1)
            usums = (us0, us1)
            for g in range(G):
                embT = psB.tile([D_EMB, N], F32, tag="ot", name=f"embT{g}")
                for i in range(NCH):
                    gat = wk.tile([128, D_EMB], F32, tag="gat", bufs=2 * NCH,
                                  name=f"gat{g}_{i}")
                    nc.gpsimd.indirect_dma_start(
                        out=gat[:], out_offset=None, in_=d_emb[:],
                        in_offset=bass.IndirectOffsetOnAxis(
                            ap=idx[:, g * NCH + i:g * NCH + i + 1], axis=0))
                    nc.tensor.transpose(embT[:, 128 * i:128 * (i + 1)], gat[:], ident[:])
                embTs.append(embT)
                norm_stats(embT[:], D_EMB, esums, g)
                norm_stats(ueT[0:3, g * N:(g + 1) * N], 3, usums, g)
            norm_finish([embTs[g][:] for g in range(G)], D_EMB, esums,
                        nwb[:, 0:1], nwb[:, 1:2],
                        [xTa[F0:128, g * N:(g + 1) * N] for g in range(G)])
            norm_finish([ueT[0:3, g * N:(g + 1) * N] for g in range(G)], 3, usums,
                        nwb[0:3, 2:3], nwb[0:3, 3:4],
                        [xTb[:, g * N:(g + 1) * N] for g in range(G)])

        # ---------- pair stages ----------
        def stage1a(l, h):
            """hp matmuls (stationary shared across graphs), tanh, bf16 copy,
            DMA-XBAR transposes into node-major 129-stride layout."""
            st = {"l": l, "h": h}
            hps = []
            for g in range(G):
                hps.append(psA.tile([128, N], F32, tag="hp", name=f"hp{l}_{h}_{g}"))
            if l == 0:
                for g in range(G):
                    MM(hps[g][:], w0a[:, 128 * h:128 * (h + 1)],
                       xTa[:, g * N:(g + 1) * N], start=True, stop=False)
                for g in range(G):
                    MM(hps[g][:], w0b[:, 128 * h:128 * (h + 1)],
                       xTb[:, g * N:(g + 1) * N], start=False, stop=True)
            else:
                for k in range(8):
                    for g in range(G):
                        MM(hps[g][:], w1[:, (h * 8 + k) * 128:(h * 8 + k + 1) * 128],
                           x1seg[g][k][:], start=(k == 0), stop=(k == 7))
            t_sbs, hp_nms = [], []
            for g in range(G):
                t_sb = wk.tile([128, N], BF16, tag="tt", name=f"t{g}")
                if l == 1:
                    nc.scalar.activation(t_sb[:], hps[g][:], ACT.Tanh,
                                         bias=ncb[:, h:h + 1])
                else:
                    nc.scalar.activation(t_sb[:], hps[g][:], ACT.Tanh)
                t_sbs.append(t_sb)
            for g in range(G):
                hp_sb = wk.tile([128, N], BF16, tag="hpsb", name=f"hpsb{g}")
                if g == 0:
                    if l == 1:
                        nc.scalar.activation(hp_sb[:], hps[g][:], ACT.Identity,
                                             bias=ncb[:, h:h + 1])
                    else:
                        nc.scalar.copy(hp_sb[:], hps[g][:])
                else:
                    if l == 1:
                        nc.vector.tensor_scalar(hp_sb[:], hps[g][:],
                                                ncb[:, h:h + 1], None, OP.add)
                    else:
                        nc.vector.tensor_copy(out=hp_sb[:], in_=hps[g][:])
                hp_nm = [xb.tile([128, 129], BF16, tag="hpnm", name=f"hpnm{g}_{j}")
                         for j in range(NCH)]
                for j in range(NCH):
                    nc.sync.dma_start_transpose(hp_nm[j][:, 0:128],
                                                hp_sb[:, 128 * j:128 * (j + 1)])
                    nc.gpsimd.memset(hp_nm[j][:, 128:129], 1.0)
                hp_nms.append(hp_nm)
            st.update(t=t_sbs, hp_nm=hp_nms)
            return st

        def stage1b(st):
            """scores: sbc broadcast matmul -> exp; d columns -> exp, exp(0.2)."""
            l, h = st["l"], st["h"]
            pcbs, qas = [], []
            for g in range(G):
                sbc = psS.tile([128, N], F32, tag="sbc", name=f"sbc{g}")
                MM(sbc[:], asr[:, (l * H + h) * 128:(l * H + h) * 128 + 128],
                   st["t"][g][:], start=True, stop=True)
                cols = psC.tile([128, NCH], F32, tag="cols", name=f"cols{g}")
                for j in range(NCH):
                    MM(cols[:, j:j + 1], st["t"][g][:, 128 * j:128 * (j + 1)],
                       adc[:, l * H + h:l * H + h + 1], start=True, stop=True)
                pcb = wk.tile([128, N], BF16, tag="pcb", name=f"pcb{g}")
                nc.scalar.activation(pcb[:], sbc[:], ACT.Exp)
                pcbs.append(pcb)
                qa = wk.tile([128, 2 * NCH], F32, tag="qa", name=f"qa{g}")
                nc.scalar.activation(qa[:, 0:NCH], cols[:], ACT.Exp)
                nc.scalar.activation(qa[:, NCH:2 * NCH], cols[:], ACT.Exp, scale=0.2)
                qas.append(qa)
            st.update(pcb=pcbs, qa=qas)
            if DBG and l == 0 and h == 0:
                nc.sync.dma_start(d_dbg_pcb[:], pcbs[0][:])
                nc.sync.dma_start(d_dbg_qa[:], qas[0][:])
                nc.sync.dma_start(d_dbg_t[:], st["t"][0][:])

        def stage2(st):
            """E' chunks: (pcb*qcol max acol) * adjT on DVE (+1 chunk on gpsimd
            for L1 pairs)."""
            l = st["l"]
            eps_ = []
            for g in range(G):
                e_g = []
                for j in range(NCH):
                    ep = ep_pool.tile([128, N], BF16, tag="ep", name=f"ep{g}_{j}")
                    nc.vector.tensor_scalar(ep[:], st["pcb"][g][:],
                                            st["qa"][g][:, j:j + 1],
                                            st["qa"][g][:, NCH + j:NCH + j + 1],
                                            OP.mult, OP.max)
                    e = ep_pool.tile([128, N], BF16, tag="ep", name=f"e{g}_{j}")
                    adj_sl = adjT[:, (g * NCH + j) * N:(g * NCH + j + 1) * N]
                    nc.vector.tensor_tensor(out=e[:], in0=ep[:], in1=adj_sl,
                                            op=OP.mult)
                    e_g.append(e)
                eps_.append(e_g)
            st["eps"] = eps_
            if DBG and st["h"] == 0 and l == 0:
                for j in range(NCH):
                    nc.sync.dma_start(d_dbg_e[:, j * N:(j + 1) * N],
                                      eps_[0][j][:])
                for jj in range(NCH):
                    nc.sync.dma_start(d_dbg_hpnm[:, jj * 129:(jj + 1) * 129],
                                      st["hp_nm"][0][jj][:])

        def stage34(st):
            """out[n,o] node-major with fused ones-column denominators, then
            normalization tails; per graph so psum frees within the stage."""
            l, h = st["l"], st["h"]
            for g in range(G):
                ot = [psB.tile([128, 258], F32, tag="ot", name=f"ot{g}_{half}")
                      for half in range(2)]
                for j in range(NCH):
                    dst = ot[j // 2][:, (j % 2) * 129:(j % 2) * 129 + 129]
                    for k in range(NCH):
                        MM(dst, st["eps"][g][k][:, 128 * j:128 * (j + 1)],
                           st["hp_nm"][g][k][:],
                           start=(k == 0), stop=(k == NCH - 1))
                rcol = wk.tile([128, NCH], F32, tag="rcol", name=f"rcol{g}")
                nc.vector.reciprocal_approx_fast(out=rcol[:, 0:2], in_=ot[0][:, 128::129])
                nc.vector.reciprocal_approx_fast(out=rcol[:, 2:4], in_=ot[1][:, 128::129])
                if l == 0:
                    # x1' = elu(out/r)+1 = min(relu(z)+1, exp(z)), z = out*rcol
                    ez = wk.tile([128, N], BF16, tag="ez", name=f"ez{g}")
                    rz = wk.tile([128, N], BF16, tag="rz", name=f"rz{g}")
                    for j in range(NCH):
                        src = ot[j // 2][:, (j % 2) * 129:(j % 2) * 129 + 128]
                        nc.scalar.activation(ez[:, 128 * j:128 * (j + 1)], src,
                                             ACT.Exp, scale=rcol[:, j:j + 1])
                        nc.vector.tensor_scalar(rz[:, 128 * j:128 * (j + 1)], src,
                                                rcol[:, j:j + 1], 0.0,
                                                OP.mult, OP.max)
                    x1c = wk.tile([128, N], BF16, tag="x1c", name=f"x1c{g}")
                    nc.vector.scalar_tensor_tensor(out=x1c[:], in0=rz[:], scalar=1.0,
                                                   in1=ez[:], op0=OP.add, op1=OP.min)
                    nc.sync.dma_start_transpose(
                        x1seg[g][h][:].rearrange("p (c q) -> p c q", c=NCH),
                        x1c[:])
                else:
                    # accn += out * rcol * (1/8), head-mean fold
                    rcol8 = wk.tile([128, NCH], F32, tag="rcol8", name=f"rcol8{g}")
                    nc.vector.tensor_scalar(rcol8[:], rcol[:], 0.125, None, OP.mult)
                    zn = wk.tile([128, N], BF16, tag="zn", name=f"zn{g}")
                    for j in range(NCH):
                        src = ot[j // 2][:, (j % 2) * 129:(j % 2) * 129 + 128]
                        if g == 0:
                            nc.scalar.activation(zn[:, 128 * j:128 * (j + 1)], src,
                                                 ACT.Identity,
                                                 scale=rcol8[:, j:j + 1])
                        else:
                            nc.vector.tensor_scalar(zn[:, 128 * j:128 * (j + 1)],
                                                    src, rcol8[:, j:j + 1], None,
                                                    OP.mult)
                    dst = accn[:, g * NCH * 128:(g + 1) * NCH * 128]
                    if h == 0:
                        nc.vector.tensor_copy(out=dst, in_=zn[:])
                    else:
                        nc.vector.tensor_tensor(out=dst, in0=dst, in1=zn[:],
                                                op=OP.add)
                    if h == H - 1:
                        epilogue_exp(g)

        # ---------- epilogue part 1 (exp domain; per graph) ----------
        nmax_all = pers.tile([128, G * NCH], F32, tag="nmax_all")
        sexp_all = pers.tile([128, G * NCH], F32, tag="sexp_all")

        def epilogue_exp(g):
            for j in range(NCH):
                c = g * NCH + j
                blk = accn[:, c * 128:(c + 1) * 128]
                nc.vector.tensor_reduce(nmax_all[:, c:c + 1], blk, AX.X, OP.max,
                                        negate=True)
                esc = wk.tile([128, 128], BF16, tag="esc", name="esc")
                nc.scalar.activation(esc[:], blk, ACT.Exp, bias=nmax_all[:, c:c + 1],
                                     accum_out=sexp_all[:, c:c + 1])

        # ---------- pipeline ----------
        pairs = [(l, h) for l in range(2) for h in range(H)]
        prep_all()
        for i in range(6):
            wu = psA.tile([128, N], F32, tag="hp", name=f"wu2_{i}")
            MM(wu[0:1, :], zeros_w[:, 0:1], zeros_w[:], start=True, stop=True)

        # slot schedule: L0 pairs, BUB empty slots, L1 pairs
        sched = list(range(H)) + [None] * BUB + list(range(H, 2 * H))
        NS = len(sched)
        sts = {}

        def pair_at(s):
            if 0 <= s < NS and sched[s] is not None:
                return sched[s]
            return None

        for s in range(NS + 2):
            pb = pair_at(s - 1)
            pc = pair_at(s - 2)
            pa = pair_at(s)
            if pb is not None:
                stage1b(sts[pb])
                stage2(sts[pb])
            if pc is not None:
                stage34(sts[pc])
            if pa is not None:
                sts[pa] = stage1a(*pairs[pa])

        if DBG:
            for g in range(G):
                for k in range(8):
                    nc.sync.dma_start(d_dbg_x1T[:, (g * 8 + k) * N:(g * 8 + k + 1) * N],
                                      x1seg[g][k][:])
            nc.sync.dma_start(d_dbg_accn[:], accn[:])
            nc.sync.dma_start(d_dbg_xta[:], xTa[:])

        # ---------- epilogue part 2: ONE Ln + final add + output DMA ----------
        lns = pers.tile([128, G * NCH], F32, tag="lns")
        nc.scalar.activation(lns[:], sexp_all[:], ACT.Ln)
        cc = pers.tile([128, G * NCH], F32, tag="cc")
        nc.vector.tensor_tensor(out=cc[:], in0=nmax_all[:], in1=lns[:],
                                op=OP.subtract)
        for g in range(G):
            fin = wk.tile([128, 4 * 128], F32, tag="fin", bufs=2, name="fin")
            for j in range(NCH):
                i = g * NCH + j
                blk = accn[:, i * 128:(i + 1) * 128]
                if j % 2 == 0:
                    nc.vector.tensor_scalar(fin[:, j * 128:(j + 1) * 128], blk,
                                            cc[:, i:i + 1], None, OP.add)
                else:
                    nc.scalar.activation(fin[:, j * 128:(j + 1) * 128], blk,
                                         ACT.Identity, bias=cc[:, i:i + 1])
            fin_v = fin[:].rearrange("p (j o) -> p j o", j=NCH)
            nc.scalar.dma_start(d_out[g].rearrange("(j p) o -> p j o", j=NCH), fin_v)

    nc.finalize()
    return nc


def _get_nc():
    if "nc" not in _cache:
        _cache["nc"] = _build_nc()
    return _cache["nc"]


def shard_inputs(inputs):
    """Full inputs -> list of 8 per-core input maps (pure layout/dtype prep)."""
    import ml_dtypes
    bf16 = ml_dtypes.bfloat16

    adj = np.asarray(inputs["adj"], dtype=np.float32)
    h = np.asarray(inputs["h"], dtype=np.float32)
    ue = np.asarray(inputs["user_emb"], dtype=np.float32)
    emb = np.ascontiguousarray(np.asarray(inputs["emb_table"], dtype=np.float32))
    vert = np.asarray(inputs["vertices"]).astype(np.int32)

    w0 = np.asarray(inputs["w0"], dtype=np.float32)
    w0a = np.ascontiguousarray(
        w0[:, :128, :].transpose(1, 0, 2).reshape(128, H * 128).astype(bf16))
    w0b = np.ascontiguousarray(
        w0[:, 128:131, :].transpose(1, 0, 2).reshape(3, H * 128).astype(bf16))
    w1 = np.asarray(inputs["w1"], dtype=np.float32)
    w1p = np.ascontiguousarray(
        w1.reshape(H, 8, 128, 128).transpose(2, 0, 1, 3).reshape(128, H * 8 * 128)
        .astype(bf16))
    # asr: replicated 0.8*a_src columns, [o, (l*H+h)*128 + j] for all j
    asr = np.zeros((128, 2 * H * 128), np.float32)
    adc = np.zeros((128, 2 * H), np.float32)
    for h_ in range(H):
        asr[:, (0 * H + h_) * 128:(0 * H + h_ + 1) * 128] = \
            (0.8 * np.asarray(inputs["a_src0"])[h_, :, 0])[:, None]
        asr[:, (1 * H + h_) * 128:(1 * H + h_ + 1) * 128] = \
            (0.8 * np.asarray(inputs["a_src1"])[h_, :, 0])[:, None]
        adc[:, 0 * H + h_] = np.asarray(inputs["a_dst0"])[h_, :, 0]
        adc[:, 1 * H + h_] = np.asarray(inputs["a_dst1"])[h_, :, 0]
    asr = asr.astype(bf16)
    adc = adc.astype(bf16)
    # ncb: -sum_f w1[h,f,o] (elu-1 fold correction), applied as bias pre-tanh
    # and pre-copy on layer-1 hp. Note w1 here is the bf16-rounded weight.
    w1b = w1p.astype(np.float32).reshape(128, H, 8, 128)
    ncb = np.zeros((128, H), np.float32)
    for h_ in range(H):
        ncb[:, h_] = -w1b[:, h_, :, :].sum(axis=(0, 1))
    nw = np.zeros((D_EMB, 4), np.float32)
    nw[:, 0] = np.asarray(inputs["norm1_w"], dtype=np.float32)
    nw[:, 1] = np.asarray(inputs["norm1_b"], dtype=np.float32)
    nw[0:3, 2] = np.asarray(inputs["norm2_w"], dtype=np.float32)
    nw[0:3, 3] = np.asarray(inputs["norm2_b"], dtype=np.float32)

    maps = []
    for c in range(NCORES):
        sl = slice(G * c, G * (c + 1))
        adjT = adj[sl].transpose(0, 2, 1).reshape(G, NCH, 128, N) \
            .transpose(2, 0, 1, 3).reshape(128, G * NCH * N).astype(bf16)
        hT = h[sl].transpose(0, 2, 1).transpose(1, 0, 2).reshape(F0, G * N).astype(bf16)
        ueT = ue[sl].transpose(0, 2, 1).transpose(1, 0, 2).reshape(3, G * N)
        idxp = vert[sl].reshape(G, NCH, 128).transpose(2, 0, 1).reshape(128, G * NCH)
        maps.append({
            "adjT": np.ascontiguousarray(adjT),
            "hT": np.ascontiguousarray(hT),
            "ueT": np.ascontiguousarray(ueT.astype(np.float32)),
            "idx": np.ascontiguousarray(idxp),
            "emb": emb,
            "w0a": w0a, "w0b": w0b, "w1": w1p,
            "asr": asr, "adc": adc, "ncb": ncb, "nw": nw,
        })
    return maps


def kernel(**inputs):
    _ensure_paths()
    from concourse import bass_utils
    nc = _get_nc()
    maps = shard_inputs(inputs)
    res = bass_utils.run_bass_kernel_spmd(nc, maps, core_ids=list(range(NCORES)))
    out = np.concatenate([res.results[c]["out"] for c in range(NCORES)], axis=0)
    return out


# revision 39
# speedup vs baseline: 1.4359x; 1.1259x over previous
"""Trainium2 Bass kernel for nn_BatchdenseGAT: 2-layer dense GAT, batch 16x512 nodes.

v3: data-parallel (2 graphs/core), 16 (layer,head) pairs each covering both
graphs (stationary weights shared across graphs). Major changes vs v2:
  - exp(0.8 s) is produced BROADCAST across partitions by a matmul whose
    stationary is 0.8*a_src replicated into 128 columns (sbc psum -> one ACT
    exp). No [1,n] rows, no PE rank-1 outer products.
  - E'[m,n] = adjT * max(pcb*exp(d[m]), exp(0.2 d[m])) built on DVE:
    tensor_scalar (mult,max; 4x mode) + tensor_tensor mask-mult (2x mode).
  - all hp/x1 transposes go through DMA-XBAR (dma_start_transpose), PE does
    zero transposes in the main loop.
  - elu(z)-1 fold: layer-1 consumes x1' = elu(z)+1; the -1 is folded into a
    per-feature bias (host-precomputed w1 column sums) applied in tanh/copy.
    L0 tail: x1' = min(relu(z)+1, exp(z)) = 2 cheap ops + one gpsimd stt.
  - out matmul keeps node-major layout with the fused ones-column (softmax
    denominator at column 128 of each 129-wide mov block).
  - software pipeline issues oldest-stage-first per slot so no engine queue
    head-of-line blocks; 3 drain slots between layer 0 and layer 1.
"""

import os
import sys
import numpy as np

B, N, V, D_EMB, F0, H = 16, 512, 100000, 64, 64, 8
O1 = O2 = 128
EPS = 1e-5
NCORES = 8
G = B // NCORES         # graphs per core = 2
NCH = N // 128          # 4 node chunks

_cache = {}


def _ensure_paths():
    p = "/opt/trn_rl_repo/concourse"
    if os.path.isdir(p) and p not in sys.path:
        sys.path.append(p)


N_WARMUP = 14           # PE warmup matmuls (p-state ramp) during input DMA
BUB = 2                 # drain slots between layer 0 and layer 1
DBG = os.environ.get("KDBG", "0") == "1"


def _build_nc():
    _ensure_paths()
    import concourse.bass as bass
    import concourse.tile as tile
    import concourse.mybir as mybir
    from concourse import bacc
    from contextlib import ExitStack

    F32 = mybir.dt.float32
    BF16 = mybir.dt.bfloat16
    I32 = mybir.dt.int32
    AX = mybir.AxisListType
    OP = mybir.AluOpType
    ACT = mybir.ActivationFunctionType

    nc = bacc.Bacc("TRN2", debug=False, enable_asserts=False)

    d_adjT = nc.dram_tensor("adjT", [128, G * NCH * N], BF16, kind="ExternalInput").ap()
    d_hT = nc.dram_tensor("hT", [F0, G * N], BF16, kind="ExternalInput").ap()
    d_ueT = nc.dram_tensor("ueT", [3, G * N], F32, kind="ExternalInput").ap()
    d_idx = nc.dram_tensor("idx", [128, G * NCH], I32, kind="ExternalInput").ap()
    d_emb = nc.dram_tensor("emb", [V, D_EMB], F32, kind="ExternalInput").ap()
    d_w0a = nc.dram_tensor("w0a", [128, H * 128], BF16, kind="ExternalInput").ap()
    d_w0b = nc.dram_tensor("w0b", [3, H * 128], BF16, kind="ExternalInput").ap()
    d_w1 = nc.dram_tensor("w1", [128, H * 8 * 128], BF16, kind="ExternalInput").ap()
    d_asr = nc.dram_tensor("asr", [128, 2 * H * 128], BF16, kind="ExternalInput").ap()
    d_adc = nc.dram_tensor("adc", [128, 2 * H], BF16, kind="ExternalInput").ap()
    d_ncb = nc.dram_tensor("ncb", [128, H], F32, kind="ExternalInput").ap()
    d_nw = nc.dram_tensor("nw", [D_EMB, 4], F32, kind="ExternalInput").ap()
    d_out = nc.dram_tensor("out", [G, N, O2], F32, kind="ExternalOutput").ap()
    if DBG:
        d_dbg_x1T = nc.dram_tensor("dbg_x1T", [128, G * 8 * N], BF16,
                                   kind="ExternalOutput").ap()
        d_dbg_accn = nc.dram_tensor("dbg_accn", [128, G * NCH * 128], BF16,
                                    kind="ExternalOutput").ap()
        d_dbg_hpnm = nc.dram_tensor("dbg_hpnm", [128, NCH * 129], BF16,
                                    kind="ExternalOutput").ap()
        d_dbg_e = nc.dram_tensor("dbg_e", [128, NCH * N], BF16,
                                 kind="ExternalOutput").ap()
        d_dbg_pcb = nc.dram_tensor("dbg_pcb", [128, N], BF16,
                                   kind="ExternalOutput").ap()
        d_dbg_qa = nc.dram_tensor("dbg_qa", [128, 2 * NCH], F32,
                                  kind="ExternalOutput").ap()
        d_dbg_t = nc.dram_tensor("dbg_t", [128, N], BF16,
                                 kind="ExternalOutput").ap()
        d_dbg_xta = nc.dram_tensor("dbg_xta", [128, G * N], BF16,
                                   kind="ExternalOutput").ap()

    with tile.TileContext(nc) as tc, ExitStack() as ctx:
        pers = ctx.enter_context(tc.tile_pool(name="pers", bufs=1))
        wk = ctx.enter_context(tc.tile_pool(name="wk", bufs=4))
        xb = ctx.enter_context(tc.tile_pool(name="xb", bufs=24))
        ep_pool = ctx.enter_context(tc.tile_pool(name="ep", bufs=40))
        psA = ctx.enter_context(tc.tile_pool(name="psA", bufs=2, space="PSUM"))
        psS = ctx.enter_context(tc.tile_pool(name="psS", bufs=2, space="PSUM"))
        psB = ctx.enter_context(tc.tile_pool(name="psB", bufs=3, space="PSUM"))
        psC = ctx.enter_context(tc.tile_pool(name="psC", bufs=1, space="PSUM"))

        MM = nc.tensor.matmul

        # ---------- persistents + input DMAs ----------
        adjT = pers.tile([128, G * NCH * N], BF16, tag="adjT")
        xTa = pers.tile([128, G * N], BF16, tag="xTa")
        xTb = pers.tile([3, G * N], BF16, tag="xTb")
        ueT = pers.tile([3, G * N], F32, tag="ueT")
        x1seg = [[pers.tile([128, N], BF16, tag=f"x1seg{g}_{k}",
                            name=f"x1seg{g}_{k}")
                  for k in range(8)] for g in range(G)]
        accn = pers.tile([128, G * NCH * 128], BF16, tag="accn")
        w0a = pers.tile([128, H * 128], BF16, tag="w0a")
        w0b = pers.tile([3, H * 128], BF16, tag="w0b")
        w1 = pers.tile([128, H * 8 * 128], BF16, tag="w1")
        asr = pers.tile([128, 2 * H * 128], BF16, tag="asr")
        adc = pers.tile([128, 2 * H], BF16, tag="adc")
        ncb = pers.tile([128, H], F32, tag="ncb")
        nwb = pers.tile([D_EMB, 4], F32, tag="nwb")
        idx = pers.tile([128, G * NCH], I32, tag="idx")
        zeros_w = pers.tile([128, N], BF16, tag="zeros_w")

        nc.vector.memset(zeros_w[:], 0.0)

        nc.sync.dma_start(idx[:], d_idx[:])
        nc.scalar.dma_start(nwb[:], d_nw[:])
        nc.sync.dma_start(ueT[:], d_ueT[:])
        nc.sync.dma_start(xTa[0:F0, :], d_hT[:])
        nc.scalar.dma_start(w0a[:], d_w0a[:])
        nc.scalar.dma_start(w0b[:], d_w0b[:])
        nc.scalar.dma_start(asr[:], d_asr[:])
        nc.scalar.dma_start(adc[:], d_adc[:])
        nc.scalar.dma_start(ncb[:], d_ncb[:])
        nc.sync.dma_start(adjT[:], d_adjT[:])
        nc.scalar.dma_start(w1[:], d_w1[:])

        # ---------- PE warmup: keep p-state high while DMAs land ----------
        for i in range(N_WARMUP):
            wu = psA.tile([128, N], F32, tag="hp", name=f"wu{i}")
            MM(wu[0:1, :], zeros_w[:, 0:1], zeros_w[:], start=True, stop=True)

        # ---------- per-graph preprocessing (instance norms + embed) ----------
        ident = pers.tile([128, 128], F32, tag="ident")
        ident_b = pers.tile([128, 128], BF16, tag="ident_b")
        from concourse.masks import make_identity
        make_identity(nc, ident[:])
        make_identity(nc, ident_b[:])

        def norm_stats(src, P, sums, col):
            nc.vector.tensor_reduce(sums[0][:, col:col + 1], src, AX.X, OP.add)
            sq = wk.tile([P, N], BF16, tag="in_sq", name=f"sq{col}")
            nc.scalar.activation(sq[:], src, ACT.Square,
                                 accum_out=sums[1][:, col:col + 1])

        def norm_finish(srcs, P, sums, w_col, b_col, dsts):
            mu = wk.tile([P, G], F32, tag="in_mu")
            nc.vector.tensor_scalar(mu[:], sums[0][:], 1.0 / N, None, OP.mult)
            ex2 = wk.tile([P, G], F32, tag="in_ex2")
            nc.vector.tensor_scalar(ex2[:], sums[1][:], 1.0 / N, None, OP.mult)
            musq = wk.tile([P, G], F32, tag="in_musq")
            nc.vector.tensor_tensor(out=musq[:], in0=mu[:], in1=mu[:], op=OP.mult)
            vare = wk.tile([P, G], F32, tag="in_vare")
            nc.vector.tensor_tensor(out=vare[:], in0=ex2[:], in1=musq[:], op=OP.subtract)
            nc.vector.tensor_scalar(vare[:], vare[:], EPS, None, OP.add)
            iv = vare[:].bitcast(mybir.dt.int32)
            sh = wk.tile([P, G], mybir.dt.int32, tag="in_sh")
            nc.vector.tensor_scalar(sh[:], iv, 1, None, OP.arith_shift_right)
            y = wk.tile([P, G], F32, tag="in_y")
            nc.vector.tensor_scalar(y[:].bitcast(mybir.dt.int32), sh[:], -1,
                                    0x5f3759df, OP.mult, OP.add)
            rstd = y
            for it in range(2):
                y2 = wk.tile([P, G], F32, tag="in_y2", name=f"y2{it}")
                nc.vector.tensor_tensor(out=y2[:], in0=rstd[:], in1=rstd[:], op=OP.mult)
                vy2 = wk.tile([P, G], F32, tag="in_vy2", name=f"vy2{it}")
                nc.vector.tensor_tensor(out=vy2[:], in0=vare[:], in1=y2[:], op=OP.mult)
                corr = wk.tile([P, G], F32, tag="in_corr", name=f"corr{it}")
                nc.vector.tensor_scalar(corr[:], vy2[:], -0.5, 1.5, OP.mult, OP.add)
                ynew = wk.tile([P, G], F32, tag="in_ynew", name=f"ynew{it}")
                nc.vector.tensor_tensor(out=ynew[:], in0=rstd[:], in1=corr[:], op=OP.mult)
                rstd = ynew
            scl = wk.tile([P, G], F32, tag="in_scl")
            nc.vector.tensor_scalar(scl[:], rstd[:], w_col, None, OP.mult)
            tb = wk.tile([P, G], F32, tag="in_tb")
            nc.vector.tensor_tensor(out=tb[:], in0=mu[:], in1=scl[:], op=OP.mult)
            bia = wk.tile([P, G], F32, tag="in_bia")
            nc.vector.tensor_scalar(bia[:], tb[:], -1.0, b_col, OP.mult, OP.add)
            for g in range(G):
                nc.vector.tensor_scalar(dsts[g], srcs[g], scl[:, g:g + 1],
                                        bia[:, g:g + 1], OP.mult, OP.add)

        def prep_all():
            embTs = []
            es0 = pers.tile([D_EMB, G], F32, tag="es0", name="es0")
            es1 = pers.tile([D_EMB, G], F32, tag="es1", name="es1")
            us0 = pers.tile([3, G], F32, tag="us0", name="us0")
            us1 = pers.tile([3, G], F32, tag="us1", name="us1")
            esums = (es0, es1)
            usums = (us0, us1)
            for g in range(G):
                embT = psB.tile([D_EMB, N], F32, tag="ot", name=f"embT{g}")
                for i in range(NCH):
                    gat = wk.tile([128, D_EMB], F32, tag="gat", bufs=2 * NCH,
                                  name=f"gat{g}_{i}")
                    nc.gpsimd.indirect_dma_start(
                        out=gat[:], out_offset=None, in_=d_emb[:],
                        in_offset=bass.IndirectOffsetOnAxis(
                            ap=idx[:, g * NCH + i:g * NCH + i + 1], axis=0))
                    nc.tensor.transpose(embT[:, 128 * i:128 * (i + 1)], gat[:], ident[:])
                embTs.append(embT)
                norm_stats(embT[:], D_EMB, esums, g)
                norm_stats(ueT[0:3, g * N:(g + 1) * N], 3, usums, g)
            norm_finish([embTs[g][:] for g in range(G)], D_EMB, esums,
                        nwb[:, 0:1], nwb[:, 1:2],
                        [xTa[F0:128, g * N:(g + 1) * N] for g in range(G)])
            norm_finish([ueT[0:3, g * N:(g + 1) * N] for g in range(G)], 3, usums,
                        nwb[0:3, 2:3], nwb[0:3, 3:4],
                        [xTb[:, g * N:(g + 1) * N] for g in range(G)])

        # ---------- pair stages ----------
        def stage1a(l, h):
            """hp matmuls (stationary shared across graphs), tanh, bf16 copy,
            DMA-XBAR transposes into node-major 129-stride layout."""
            st = {"l": l, "h": h}
            hps = []
            for g in range(G):
                hps.append(psA.tile([128, N], F32, tag="hp", name=f"hp{l}_{h}_{g}"))
            if l == 0:
                for g in range(G):
                    MM(hps[g][:], w0a[:, 128 * h:128 * (h + 1)],
                       xTa[:, g * N:(g + 1) * N], start=True, stop=False)
                for g in range(G):
                    MM(hps[g][:], w0b[:, 128 * h:128 * (h + 1)],
                       xTb[:, g * N:(g + 1) * N], start=False, stop=True)
            else:
                for k in range(8):
                    for g in range(G):
                        MM(hps[g][:], w1[:, (h * 8 + k) * 128:(h * 8 + k + 1) * 128],
                           x1seg[g][k][:], start=(k == 0), stop=(k == 7))
            t_sbs, hp_nms = [], []
            for g in range(G):
                t_sb = wk.tile([128, N], BF16, tag="tt", name=f"t{g}")
                if l == 1:
                    nc.scalar.activation(t_sb[:], hps[g][:], ACT.Tanh,
                                         bias=ncb[:, h:h + 1])
                else:
                    nc.scalar.activation(t_sb[:], hps[g][:], ACT.Tanh)
                t_sbs.append(t_sb)
            for g in range(G):
                hp_sb = wk.tile([128, N], BF16, tag="hpsb", name=f"hpsb{g}")
                if g == 0:
                    if l == 1:
                        nc.scalar.activation(hp_sb[:], hps[g][:], ACT.Identity,
                                             bias=ncb[:, h:h + 1])
                    else:
                        nc.scalar.copy(hp_sb[:], hps[g][:])
                else:
                    if l == 1:
                        nc.vector.tensor_scalar(hp_sb[:], hps[g][:],
                                                ncb[:, h:h + 1], None, OP.add)
                    else:
                        nc.vector.tensor_copy(out=hp_sb[:], in_=hps[g][:])
                hp_nm4 = xb.tile([128, NCH, 128], BF16, tag="hpnm4",
                                 name=f"hpnm4{g}")
                nc.sync.dma_start_transpose(hp_nm4[:], hp_sb[:])
                hp_nm = xb.tile([128, NCH * 129], BF16, tag="hpnm", name=f"hpnm{g}")
                dst_v = hp_nm[:].rearrange("p (k c) -> p k c", k=NCH)[:, :, 0:128]
                if g == 0:
                    nc.scalar.copy(dst_v, hp_nm4[:])
                else:
                    nc.vector.tensor_copy(out=dst_v, in_=hp_nm4[:])
                nc.gpsimd.memset(hp_nm[:, 128::129], 1.0)
                hp_nms.append(hp_nm)
            st.update(t=t_sbs, hp_nm=hp_nms)
            return st

        def stage1b(st):
            """scores: sbc broadcast matmul -> exp; d columns -> exp, exp(0.2)."""
            l, h = st["l"], st["h"]
            pcbs, qas = [], []
            for g in range(G):
                sbc = psS.tile([128, N], F32, tag="sbc", name=f"sbc{g}")
                MM(sbc[:], asr[:, (l * H + h) * 128:(l * H + h) * 128 + 128],
                   st["t"][g][:], start=True, stop=True)
                cols = psC.tile([128, NCH], F32, tag="cols", name=f"cols{g}")
                for j in range(NCH):
                    MM(cols[:, j:j + 1], st["t"][g][:, 128 * j:128 * (j + 1)],
                       adc[:, l * H + h:l * H + h + 1], start=True, stop=True)
                pcb = wk.tile([128, N], BF16, tag="pcb", name=f"pcb{g}")
                nc.scalar.activation(pcb[:], sbc[:], ACT.Exp)
                pcbs.append(pcb)
                qa = wk.tile([128, 2 * NCH], F32, tag="qa", name=f"qa{g}")
                nc.scalar.activation(qa[:, 0:NCH], cols[:], ACT.Exp)
                nc.scalar.activation(qa[:, NCH:2 * NCH], cols[:], ACT.Exp, scale=0.2)
                qas.append(qa)
            st.update(pcb=pcbs, qa=qas)
            if DBG and l == 0 and h == 0:
                nc.sync.dma_start(d_dbg_pcb[:], pcbs[0][:])
                nc.sync.dma_start(d_dbg_qa[:], qas[0][:])
                nc.sync.dma_start(d_dbg_t[:], st["t"][0][:])

        def stage2(st):
            """E' chunks: (pcb*qcol max acol) * adjT on DVE (+1 chunk on gpsimd
            for L1 pairs)."""
            l = st["l"]
            eps_ = []
            for g in range(G):
                e_g = []
                for j in range(NCH):
                    ep = ep_pool.tile([128, N], BF16, tag="ep", name=f"ep{g}_{j}")
                    nc.vector.tensor_scalar(ep[:], st["pcb"][g][:],
                                            st["qa"][g][:, j:j + 1],
                                            st["qa"][g][:, NCH + j:NCH + j + 1],
                                            OP.mult, OP.max)
                    e = ep_pool.tile([128, N], BF16, tag="ep", name=f"e{g}_{j}")
                    adj_sl = adjT[:, (g * NCH + j) * N:(g * NCH + j + 1) * N]
                    if j == 0:
                        nc.gpsimd.tensor_tensor(out=e[:], in0=ep[:], in1=adj_sl,
                                                op=OP.mult)
                    else:
                        nc.vector.tensor_tensor(out=e[:], in0=ep[:], in1=adj_sl,
                                                op=OP.mult)
                    e_g.append(e)
                eps_.append(e_g)
            st["eps"] = eps_
            if DBG and st["h"] == 0 and l == 0:
                for j in range(NCH):
                    nc.sync.dma_start(d_dbg_e[:, j * N:(j + 1) * N],
                                      eps_[0][j][:])
                nc.sync.dma_start(d_dbg_hpnm[:], st["hp_nm"][0][:])

        def stage34(st):
            """out[n,o] node-major with fused ones-column denominators, then
            normalization tails; per graph so psum frees within the stage."""
            l, h = st["l"], st["h"]
            for g in range(G):
                ot = [psB.tile([128, 258], F32, tag="ot", name=f"ot{g}_{half}")
                      for half in range(2)]
                for j in range(NCH):
                    dst = ot[j // 2][:, (j % 2) * 129:(j % 2) * 129 + 129]
                    for k in range(NCH):
                        MM(dst, st["eps"][g][k][:, 128 * j:128 * (j + 1)],
                           st["hp_nm"][g][:, k * 129:k * 129 + 129],
                           start=(k == 0), stop=(k == NCH - 1))
                rcol = wk.tile([128, NCH], F32, tag="rcol", name=f"rcol{g}")
                nc.vector.reciprocal_approx_fast(out=rcol[:, 0:2], in_=ot[0][:, 128::129])
                nc.vector.reciprocal_approx_fast(out=rcol[:, 2:4], in_=ot[1][:, 128::129])
                if l == 0:
                    # x1' = elu(out/r)+1 = min(relu(z)+1, exp(z)), z = out*rcol
                    ez = wk.tile([128, N], BF16, tag="ez", name=f"ez{g}")
                    rz = wk.tile([128, N], BF16, tag="rz", name=f"rz{g}")
                    for j in range(NCH):
                        src = ot[j // 2][:, (j % 2) * 129:(j % 2) * 129 + 128]
                        nc.scalar.activation(ez[:, 128 * j:128 * (j + 1)], src,
                                             ACT.Exp, scale=rcol[:, j:j + 1])
                        nc.scalar.activation(rz[:, 128 * j:128 * (j + 1)], src,
                                             ACT.Relu, scale=rcol[:, j:j + 1])
                    x1c = wk.tile([128, N], BF16, tag="x1c", name=f"x1c{g}")
                    nc.vector.scalar_tensor_tensor(out=x1c[:], in0=rz[:], scalar=1.0,
                                                   in1=ez[:], op0=OP.add, op1=OP.min)
                    nc.sync.dma_start_transpose(
                        x1seg[g][h][:].rearrange("p (c q) -> p c q", c=NCH),
                        x1c[:])
                else:
                    # accn += out * rcol * (1/8), head-mean fold
                    rcol8 = wk.tile([128, NCH], F32, tag="rcol8", name=f"rcol8{g}")
                    nc.vector.tensor_scalar(rcol8[:], rcol[:], 0.125, None, OP.mult)
                    zn = wk.tile([128, N], BF16, tag="zn", name=f"zn{g}")
                    for j in range(NCH):
                        src = ot[j // 2][:, (j % 2) * 129:(j % 2) * 129 + 128]
                        if g == 0:
                            nc.scalar.activation(zn[:, 128 * j:128 * (j + 1)], src,
                                                 ACT.Identity,
                                                 scale=rcol8[:, j:j + 1])
                        else:
                            nc.vector.tensor_scalar(zn[:, 128 * j:128 * (j + 1)],
                                                    src, rcol8[:, j:j + 1], None,
                                                    OP.mult)
                    dst = accn[:, g * NCH * 128:(g + 1) * NCH * 128]
                    if h == 0:
                        nc.vector.tensor_copy(out=dst, in_=zn[:])
                    else:
                        nc.vector.tensor_tensor(out=dst, in0=dst, in1=zn[:],
                                                op=OP.add)
                    if h == H - 1:
                        epilogue_exp(g)

        # ---------- epilogue part 1 (exp domain; per graph) ----------
        nmax_all = pers.tile([128, G * NCH], F32, tag="nmax_all")
        sexp_all = pers.tile([128, G * NCH], F32, tag="sexp_all")

        def epilogue_exp(g):
            for j in range(NCH):
                c = g * NCH + j
                blk = accn[:, c * 128:(c + 1) * 128]
                nc.vector.tensor_reduce(nmax_all[:, c:c + 1], blk, AX.X, OP.max,
                                        negate=True)
                esc = wk.tile([128, 128], BF16, tag="esc", name="esc")
                nc.scalar.activation(esc[:], blk, ACT.Exp, bias=nmax_all[:, c:c + 1],
                                     accum_out=sexp_all[:, c:c + 1])

        # ---------- pipeline ----------
        pairs = [(l, h) for l in range(2) for h in range(H)]
        prep_all()
        for i in range(6):
            wu = psA.tile([128, N], F32, tag="hp", name=f"wu2_{i}")
            MM(wu[0:1, :], zeros_w[:, 0:1], zeros_w[:], start=True, stop=True)

        # slot schedule: L0 pairs, BUB empty slots, L1 pairs
        sched = list(range(H)) + [None] * BUB + list(range(H, 2 * H))
        NS = len(sched)
        sts = {}

        def pair_at(s):
            if 0 <= s < NS and sched[s] is not None:
                return sched[s]
            return None

        for s in range(NS + 2):
            pb = pair_at(s - 1)
            pc = pair_at(s - 2)
            pa = pair_at(s)
            if pb is not None:
                stage1b(sts[pb])
                stage2(sts[pb])
            if pc is not None:
                stage34(sts[pc])
            if pa is not None:
                sts[pa] = stage1a(*pairs[pa])

        if DBG:
            for g in range(G):
                for k in range(8):
                    nc.sync.dma_start(d_dbg_x1T[:, (g * 8 + k) * N:(g * 8 + k + 1) * N],
                                      x1seg[g][k][:])
            nc.sync.dma_start(d_dbg_accn[:], accn[:])
            nc.sync.dma_start(d_dbg_xta[:], xTa[:])

        # ---------- epilogue part 2: ONE Ln + final add + output DMA ----------
        lns = pers.tile([128, G * NCH], F32, tag="lns")
        nc.scalar.activation(lns[:], sexp_all[:], ACT.Ln)
        cc = pers.tile([128, G * NCH], F32, tag="cc")
        nc.vector.tensor_tensor(out=cc[:], in0=nmax_all[:], in1=lns[:],
                                op=OP.subtract)
        for g in range(G):
            fin = wk.tile([128, 4 * 128], F32, tag="fin", bufs=2, name="fin")
            for j in range(NCH):
                i = g * NCH + j
                blk = accn[:, i * 128:(i + 1) * 128]
                if j % 2 == 0:
                    nc.vector.tensor_scalar(fin[:, j * 128:(j + 1) * 128], blk,
                                            cc[:, i:i + 1], None, OP.add)
                else:
                    nc.scalar.activation(fin[:, j * 128:(j + 1) * 128], blk,
                                         ACT.Identity, bias=cc[:, i:i + 1])
            fin_v = fin[:].rearrange("p (j o) -> p j o", j=NCH)
            nc.scalar.dma_start(d_out[g].rearrange("(j p) o -> p j o", j=NCH), fin_v)

    nc.finalize()
    return nc


def _get_nc():
    if "nc" not in _cache:
        _cache["nc"] = _build_nc()
    return _cache["nc"]


def shard_inputs(inputs):
    """Full inputs -> list of 8 per-core input maps (pure layout/dtype prep)."""
    import ml_dtypes
    bf16 = ml_dtypes.bfloat16

    adj = np.asarray(inputs["adj"], dtype=np.float32)
    h = np.asarray(inputs["h"], dtype=np.float32)
    ue = np.asarray(inputs["user_emb"], dtype=np.float32)
    emb = np.ascontiguousarray(np.asarray(inputs["emb_table"], dtype=np.float32))
    vert = np.asarray(inputs["vertices"]).astype(np.int32)

    w0 = np.asarray(inputs["w0"], dtype=np.float32)
    w0a = np.ascontiguousarray(
        w0[:, :128, :].transpose(1, 0, 2).reshape(128, H * 128).astype(bf16))
    w0b = np.ascontiguousarray(
        w0[:, 128:131, :].transpose(1, 0, 2).reshape(3, H * 128).astype(bf16))
    w1 = np.asarray(inputs["w1"], dtype=np.float32)
    w1p = np.ascontiguousarray(
        w1.reshape(H, 8, 128, 128).transpose(2, 0, 1, 3).reshape(128, H * 8 * 128)
        .astype(bf16))
    # asr: replicated 0.8*a_src columns, [o, (l*H+h)*128 + j] for all j
    asr = np.zeros((128, 2 * H * 128), np.float32)
    adc = np.zeros((128, 2 * H), np.float32)
    for h_ in range(H):
        asr[:, (0 * H + h_) * 128:(0 * H + h_ + 1) * 128] = \
            (0.8 * np.asarray(inputs["a_src0"])[h_, :, 0])[:, None]
        asr[:, (1 * H + h_) * 128:(1 * H + h_ + 1) * 128] = \
            (0.8 * np.asarray(inputs["a_src1"])[h_, :, 0])[:, None]
        adc[:, 0 * H + h_] = np.asarray(inputs["a_dst0"])[h_, :, 0]
        adc[:, 1 * H + h_] = np.asarray(inputs["a_dst1"])[h_, :, 0]
    asr = asr.astype(bf16)
    adc = adc.astype(bf16)
    # ncb: -sum_f w1[h,f,o] (elu-1 fold correction), applied as bias pre-tanh
    # and pre-copy on layer-1 hp. Note w1 here is the bf16-rounded weight.
    w1b = w1p.astype(np.float32).reshape(128, H, 8, 128)
    ncb = np.zeros((128, H), np.float32)
    for h_ in range(H):
        ncb[:, h_] = -w1b[:, h_, :, :].sum(axis=(0, 1))
    nw = np.zeros((D_EMB, 4), np.float32)
    nw[:, 0] = np.asarray(inputs["norm1_w"], dtype=np.float32)
    nw[:, 1] = np.asarray(inputs["norm1_b"], dtype=np.float32)
    nw[0:3, 2] = np.asarray(inputs["norm2_w"], dtype=np.float32)
    nw[0:3, 3] = np.asarray(inputs["norm2_b"], dtype=np.float32)

    maps = []
    for c in range(NCORES):
        sl = slice(G * c, G * (c + 1))
        adjT = adj[sl].transpose(0, 2, 1).reshape(G, NCH, 128, N) \
            .transpose(2, 0, 1, 3).reshape(128, G * NCH * N).astype(bf16)
        hT = h[sl].transpose(0, 2, 1).transpose(1, 0, 2).reshape(F0, G * N).astype(bf16)
        ueT = ue[sl].transpose(0, 2, 1).transpose(1, 0, 2).reshape(3, G * N)
        idxp = vert[sl].reshape(G, NCH, 128).transpose(2, 0, 1).reshape(128, G * NCH)
        maps.append({
            "adjT": np.ascontiguousarray(adjT),
            "hT": np.ascontiguousarray(hT),
            "ueT": np.ascontiguousarray(ueT.astype(np.float32)),
            "idx": np.ascontiguousarray(idxp),
            "emb": emb,
            "w0a": w0a, "w0b": w0b, "w1": w1p,
            "asr": asr, "adc": adc, "ncb": ncb, "nw": nw,
        })
    return maps


def kernel(**inputs):
    _ensure_paths()
    from concourse import bass_utils
    nc = _get_nc()
    maps = shard_inputs(inputs)
    res = bass_utils.run_bass_kernel_spmd(nc, maps, core_ids=list(range(NCORES)))
    out = np.concatenate([res.results[c]["out"] for c in range(NCORES)], axis=0)
    return out
